# revision 33
# baseline (speedup 1.0000x reference)
"""Dilated attention Trainium2 kernel (cross-rate score dedup + tuned schedule).

Problem: for each (batch, segment) pair, and each dilation rate r in {1,2,4,8}:
  q = Q_seg[::r], k = K_seg[::r], v = V_seg[::r]
  out_seg[::r] += softmax(q @ k.T) @ v        (no 1/sqrt(d) scaling)

Sharding: B=2 x n_seg=4 = 8 independent (batch, segment) pairs -> one per core.

Host-side permutation (as in v3): tokens reordered as
[j%8==0 | j%8==4 | j%4==2 | j odd], so the rate-r token set is the PREFIX
[0, 2048/r).

v4 upgrade: since S[i,j] = q_i . k_j is rate independent, rate r's score
matrix is exactly the leading (2048/r)^2 block of rate 1's.  Per q-tile the
2048-wide score row-block is computed ONCE, exp'ed ONCE (block-local row
maxes over 5 column groups: 256,256,512,512,512), and ALL applicable rates'
softmax normalizations fold into one per-(row,group) weight:
    out[i] = sum_j E[i,j] * W[i, g(j)] * v_j,
    W[i,g] = sum_{rates r covering row i and group g} exp(m_g - m_r) / Z_r
One transpose+PV chain per q-tile yields the summed-over-rates output.
PE work drops ~25% vs v3.

v5 upgrades (scheduling only):
  - Host packs Q/K/V so each SBUF destination loads with ONE wide dma_start
    (2-8KB per partition row): ~22 input DMAs instead of ~112.  Kills the
    DMA-issue serialization at the head (~570ns per dma_start on the issue
    engine) and lands KT block 0 several us earlier.
  - PSUM->SBUF output eviction of PV n-1 is emitted mid-PV-n (after the
    kt==5 transpose-copy), so the o_psum bank recycles one full PV before
    the PV that reuses it and its copies slot into DVE/ACT idle time.  The
    old len>1 deferral freed the bank only inside the PV that needed it
    (~1us PE gap per PV in the tail).  (GPSIMD cannot access PSUM on this
    target, so evictions stay on DVE/ACT.)
  - Each pair's PVs are emitted BEFORE the pair's softmax-finalize ops so
    the PV copy chain never queues behind finalize work on DVE/ACT.
"""

import sys

if "/opt/trn_rl_repo" not in sys.path:
    sys.path.insert(0, "/opt/trn_rl_repo")

import numpy as np

import concourse.bass as bass
import concourse.mybir as mybir
from concourse import tile
from concourse.masks import make_identity
from concourse.bass_utils import run_bass_kernel_spmd

SEG_LEN = 2048
D = 1024
P = 128
NDCH = D // P  # 8 d-chunks of 128
BL = 512  # score block (PSUM bank) width
NBLK = SEG_LEN // BL  # 4
NKT = SEG_LEN // P  # 16 k-tiles
F16 = mybir.dt.float16
F32 = mybir.dt.float32

# column groups for block-local softmax stats (rate boundaries 256/512/1024)
GROUPS = ((0, 256), (256, 512), (512, 1024), (1024, 1536), (1536, 2048))
NGROUP = {1: 5, 2: 3, 4: 2, 8: 1}  # prefix group count per rate

# token permutation: rate-r set {j : j % r == 0} -> prefix [0, 2048/r)
PERM = np.concatenate(
    [
        np.arange(0, SEG_LEN, 8),
        np.arange(4, SEG_LEN, 8),
        np.arange(2, SEG_LEN, 4),
        np.arange(1, SEG_LEN, 2),
    ]
)


# rates contributing to output q-tile t: prefix nesting
def tile_rates(t):
    rates = [1]
    if t < 8:
        rates.append(2)
    if t < 4:
        rates.append(4)
    if t < 2:
        rates.append(8)
    return rates


_ws_ctr = [0]


def _split_multi_waits(nc):
    """walrus in this env accepts only ONE sync-wait per instruction; move
    extras onto same-engine NoOps inserted right before the instruction."""
    for f in nc.m.functions:
        for b in f.blocks:
            out, changed = [], False
            for inst in b.instructions:
                si = inst.sync_info
                if si is not None and si.on_wait and len(si.on_wait) > 1:
                    waits = list(si.on_wait)
                    for w in waits[:-1]:
                        nop = mybir.InstNoOp(
                            name=f"waitsplit_{_ws_ctr[0]}", ins=[], outs=[]
                        )
                        _ws_ctr[0] += 1
                        nop.engine = inst.engine
                        nop.sync_info = mybir.SyncInfo(on_wait=[w], on_update=[])
                        out.append(nop)
                    si.on_wait = [waits[-1]]
                    changed = True
                out.append(inst)
            if changed:
                b.instructions = out


def build_kernel():
    # note: --enable-ldw-opt=true crashes the device (NRT_EXEC_UNIT_UNRECOVERABLE)
    # note: nc.scalar-issued xbar-transpose DMAs return wrong data in this env
    nc = bass.Bass()
    # packed layouts (see make_in_maps):
    #   Qp[r, t, c, j] = Q^T[c*128+r, t*128+j]   (tile-major q columns)
    #   Kp[r, b, c, j] = K^T[c*128+r, b*512+j]   (block-major k columns)
    #   Vp[r, kt, j]   = V[kt*128+r, j]
    Qp = nc.dram_tensor("QP", (P, NKT, NDCH, P), F16, kind="ExternalInput")
    Kp = nc.dram_tensor("KP", (P, NBLK, NDCH, BL), F16, kind="ExternalInput")
    Vp = nc.dram_tensor("VP", (P, NKT, D), F16, kind="ExternalInput")
    O = nc.dram_tensor("O", (SEG_LEN, D), F32, kind="ExternalOutput")

    Exp = mybir.ActivationFunctionType.Exp
    AX = mybir.AxisListType.X
    MAX = mybir.AluOpType.max
    MIN = mybir.AluOpType.min
    MULT = mybir.AluOpType.mult
    ADD = mybir.AluOpType.add

    with tile.TileContext(nc) as tc:
        with (
            tc.tile_pool(name="qkt", bufs=1) as qkt_pool,
            tc.tile_pool(name="pp", bufs=10) as p_pool,
            tc.tile_pool(name="pt", bufs=18) as pt_pool,
            tc.tile_pool(name="op", bufs=6) as o_pool,
            tc.tile_pool(name="st", bufs=2) as stat_pool,
            tc.tile_pool(name="spsum", bufs=2, space="PSUM") as s_psum,
            tc.tile_pool(name="ptpsum", bufs=3, space="PSUM") as pt_psum,
            tc.tile_pool(name="opsum", bufs=3, space="PSUM") as o_psum,
        ):
            QTs = qkt_pool.tile([P, NKT, NDCH, P], F16, tag="QT", name="QTs")
            KTs = qkt_pool.tile([P, NBLK, NDCH, BL], F16, tag="KT", name="KTs")
            Vt = qkt_pool.tile([P, NKT, D], F16, tag="V", name="Vt")

            # ---- input DMA program: few, wide dma_starts.  sync carries the
            # KT stream, gpsimd the QT stream + V; scalar stays free for exps.
            # priority: KT block 0 + first 4 q-tiles (head), then KT blocks,
            # then next q-tiles, then V, then remaining q-tiles.
            # KT block 0 in quarters (2 d-chunks each) so the first score
            # chain's early matmuls start as soon as their chunks land;
            # subtile deps let matmul d wait only on its own quarter
            nc.sync.dma_start(KTs[:, 0, 0:2], Kp[:, 0, 0:2])
            nc.scalar.dma_start(KTs[:, 0, 2:4], Kp[:, 0, 2:4])
            nc.sync.dma_start(KTs[:, 0, 4:6], Kp[:, 0, 4:6])
            nc.scalar.dma_start(KTs[:, 0, 6:8], Kp[:, 0, 6:8])
            for t in (15, 14, 13, 12):
                nc.gpsimd.dma_start(QTs[:, t], Qp[:, t])
            for b in (1, 2, 3):
                nc.sync.dma_start(KTs[:, b], Kp[:, b])
            for t in (11, 10, 9, 8):
                nc.gpsimd.dma_start(QTs[:, t], Qp[:, t])
            # V in 4 parts, alternating queues
            for i in range(4):
                e = nc.sync if i % 2 == 0 else nc.gpsimd
                e.dma_start(Vt[:, 4 * i : 4 * i + 4, :], Vp[:, 4 * i : 4 * i + 4, :])
            for i, t in enumerate((7, 6, 5, 4, 3, 2, 1, 0)):
                e = nc.sync if i % 2 == 0 else nc.gpsimd
                e.dma_start(QTs[:, t], Qp[:, t])

            # identity (for PE transposes, needed ~55us in) + exp table warm
            ident16 = qkt_pool.tile([P, P], F16, tag="ident", name="ident16")
            make_identity(nc, ident16[:])
            warm_in = stat_pool.tile([P, 1], F32, tag="warm_in", name="warm_in")
            nc.vector.memset(warm_in[:], 0.0)
            warm = stat_pool.tile([P, 1], F32, tag="warm", name="warm")
            nc.scalar.activation(warm[:], warm_in[:], Exp)

            # ---- score block: matmul chain + block-local max + exp
            def emit_score_block(t, b, st):
                Sb = s_psum.tile([P, BL], F32, tag="S", name="Sb")
                for d in range(NDCH):
                    nc.tensor.matmul(
                        Sb[:],
                        QTs[:, t, d, :],
                        KTs[:, b, d, :],
                        start=(d == 0),
                        stop=(d == NDCH - 1),
                    )
                ng, rs, Pt = st["ng"], st["rs"], st["Pt"]
                if b == 0:
                    # two 256-wide half groups (rate-8/4 boundaries)
                    for g in (0, 1):
                        sl = slice(g * 256, (g + 1) * 256)
                        nc.vector.tensor_reduce(
                            ng[:, g : g + 1], Sb[:, sl], AX, MAX, negate=True
                        )
                        nc.scalar.activation(
                            Pt[:, sl], Sb[:, sl], Exp,
                            bias=ng[:, g : g + 1], scale=1.0,
                            accum_out=rs[:, g : g + 1],
                        )
                else:
                    g = b + 1
                    nc.vector.tensor_reduce(
                        ng[:, g : g + 1], Sb[:], AX, MAX, negate=True
                    )
                    nc.scalar.activation(
                        Pt[:, b * BL : (b + 1) * BL], Sb[:], Exp,
                        bias=ng[:, g : g + 1], scale=1.0,
                        accum_out=rs[:, g : g + 1],
                    )

            def new_stage(t):
                return {
                    "t": t,
                    "ng": stat_pool.tile([P, 8], F32, tag="ng", bufs=6, name="ng"),
                    "rs": stat_pool.tile([P, 8], F32, tag="rs", bufs=6, name="rs"),
                    "Pt": p_pool.tile([P, SEG_LEN], F16, tag="P", name="Pt"),
                }

            # ---- finalize: per-rate Z from group stats, combined weights,
            # one rescale of Pt per group -> PV output is sum over rates,
            # already normalized
            def emit_finalize(st):
                t, ng, rs, Pt = st["t"], st["ng"], st["rs"], st["Pt"]
                rates = tile_rates(t)
                nr = len(rates)
                Z = stat_pool.tile([P, 4], F32, tag="Z", name="Z")
                cbs = {}
                for ri, r in enumerate(rates):
                    gn = NGROUP[r]
                    if gn == 1:
                        nc.vector.tensor_copy(Z[:, ri : ri + 1], rs[:, 0:1])
                        continue
                    negm = stat_pool.tile([P, 1], F32, tag="negm", name="negm")
                    nc.vector.tensor_reduce(negm[:], ng[:, :gn], AX, MIN)
                    cb = stat_pool.tile([P, 8], F32, tag=f"cb{ri}", name=f"cb{ri}")
                    nc.scalar.activation(
                        cb[:, :gn], ng[:, :gn], Exp, bias=negm[:], scale=-1.0
                    )
                    cr = stat_pool.tile([P, 8], F32, tag=f"cr{ri}", name=f"cr{ri}")
                    nc.vector.scalar_tensor_tensor(
                        cr[:, :gn], cb[:, :gn], 1.0, rs[:, :gn],
                        MULT, MULT, accum_out=Z[:, ri : ri + 1],
                    )
                    cbs[ri] = cb
                rinv = stat_pool.tile([P, 4], F32, tag="rinv", name="rinv")
                nc.vector.reciprocal(rinv[:, :nr], Z[:, :nr])
                W = stat_pool.tile([P, 8], F32, tag="W", bufs=4, name="W")
                nc.vector.tensor_scalar_mul(W[:, :5], cbs[0][:, :5], rinv[:, 0:1])
                for ri, r in list(enumerate(rates))[1:]:
                    gn = NGROUP[r]
                    if gn == 1:
                        nc.vector.tensor_scalar_add(
                            W[:, 0:1], W[:, 0:1], rinv[:, ri : ri + 1]
                        )
                    else:
                        nc.vector.scalar_tensor_tensor(
                            W[:, :gn], cbs[ri][:, :gn], rinv[:, ri : ri + 1],
                            W[:, :gn], MULT, ADD,
                        )
                for gi, (g0, g1) in enumerate(GROUPS):
                    blk = Pt[:, g0:g1]
                    if gi % 2 == 0:
                        nc.vector.tensor_scalar_mul(blk, blk, W[:, gi : gi + 1])
                    else:
                        nc.scalar.mul(blk, blk, W[:, gi : gi + 1])

            def emit_evict_half(Oh, t, n0, final=False):
                """Evict one 512-wide PSUM accumulator half to DRAM."""
                rows = slice(t * P, (t + 1) * P)
                if final:
                    # last instructions of the kernel: split 2-way to
                    # minimize the exposed post-matmul latency
                    Osb = o_pool.tile([P, BL], F32, tag="Osb", name="Osb")
                    nc.vector.tensor_copy(Osb[:, 0:256], Oh[:, 0:256])
                    nc.scalar.copy(Osb[:, 256:512], Oh[:, 256:512])
                    nc.sync.dma_start(O[rows, n0 : n0 + 256], Osb[:, 0:256])
                    nc.gpsimd.dma_start(
                        O[rows, n0 + 256 : n0 + 512], Osb[:, 256:512]
                    )
                elif n0 == 0:
                    # pass-A evict: becomes ready mid-PV, no contention
                    Osb = o_pool.tile([P, BL], F32, tag="Osb", name="Osb")
                    nc.vector.tensor_copy(Osb[:], Oh[:])
                    nc.sync.dma_start(O[rows, 0:BL], Osb[:])
                else:
                    # pass-B evict becomes ready exactly at the PV boundary;
                    # split into 128-wide copies alternating DVE/ACT so it
                    # never delays the next PV's transpose-copy chain by
                    # more than ~0.2us on either engine
                    Osb = o_pool.tile([P, BL], F32, tag="Osb", name="Osb")
                    # all quarters on ACT: DVE carries the next PV's
                    # three pre-copies at this boundary, and an eviction
                    # quarter queued ahead of them stalls the next PV's
                    # early transposes ~440ns
                    for i in range(4):
                        c = slice(i * P, (i + 1) * P)
                        nc.scalar.copy(Osb[:, c], Oh[:, c])
                    nc.gpsimd.dma_start(O[rows, n0 : n0 + BL], Osb[:])

            # ---- PV: 16 transposes + one PSUM accumulation chain over all
            # 2048 (weighted) columns, transposes running 2 ahead of PV
            def transpose_copy(Pt, kt, force_vector=False):
                ptp = pt_psum.tile([P, P], F16, tag="ptp", name="ptp")
                nc.tensor.transpose(
                    ptp[:], Pt[:, kt * P : (kt + 1) * P], ident16[:]
                )
                ptsb = pt_pool.tile([P, P], F16, tag="pts", name="pts")
                if force_vector or kt % 2 == 0:
                    nc.vector.tensor_copy(ptsb[:], ptp[:])
                else:
                    nc.scalar.copy(ptsb[:], ptp[:])
                return ptsb

            def pre_pv(st):
                """Emit the first two transpose+copies of st's PV ahead of
                time (end of the previous PV / mid-scores), so at the PV
                boundary the PE's kt2 transpose and kt0 matmul never wait on
                the DVE/ACT copy queues."""
                if st is None or "pre" in st:
                    return
                st["pre"] = [
                    transpose_copy(st["Pt"], kt, force_vector=True)
                    for kt in (0, 1, 2)
                ]

            def emit_pv(st, nxt=None, final=False):
                """Two 512-wide accumulation passes over the 16 k-chunks.
                Pass A (cols 0:512) interleaves with the transposes, pass B
                (cols 512:1024) follows; pass A's PSUM bank evicts while
                pass B computes, so with 3 o_psum banks no PV ever waits on
                an eviction, and the exposed tail is one 512-wide half."""
                t, Pt = st["t"], st["Pt"]
                OpsA = o_psum.tile([P, BL], F32, tag="O", name="OpsA")
                OpsB = o_psum.tile([P, BL], F32, tag="O", name="OpsB")

                def pv_mm(Oh, pts, kt, n0):
                    nc.tensor.matmul(
                        Oh[:],
                        pts[kt][:],
                        Vt[:, kt, n0 : n0 + BL],
                        start=(kt == 0),
                        stop=(kt == NKT - 1),
                    )

                # transposes run 4 chunks ahead of the pass-A consumers: the
                # DVE/ACT copy chain (one copy per ~272ns of PE work here)
                # needs the extra lead or pass-A matmuls stall on their pts
                pts = list(st.get("pre", ()))
                for kt in range(len(pts), NKT):
                    pts.append(transpose_copy(Pt, kt))
                    if kt >= 4:
                        pv_mm(OpsA, pts, kt - 4, 0)
                for kt in range(NKT - 4, NKT):
                    pv_mm(OpsA, pts, kt, 0)
                emit_evict_half(OpsA, t, 0)
                for kt in range(NKT):
                    pv_mm(OpsB, pts, kt, 512)
                emit_evict_half(OpsB, t, 512, final=final)
                pre_pv(nxt)

            # ---- schedule
            pending = []

            # head quad 15..12, block-major: PE progresses as KT blocks land
            quad = [new_stage(t) for t in (15, 14, 13, 12)]
            for b in range(NBLK):
                for st in quad:
                    emit_score_block(st["t"], b, st)
                    if b == NBLK - 1:
                        emit_finalize(st)
            pending.extend(quad)

            # bank one more pair (first PV waits for the full V stream);
            # its finalize is deferred into the first steady iteration's
            # scores like every later pair's
            sts = [new_stage(11), new_stage(10)]
            for b in range(NBLK):
                for st in sts:
                    emit_score_block(st["t"], b, st)
            pending.extend(sts)

            # steady state: pair scores + 2 PVs.  A pair's finalize is
            # DEFERRED into the middle of the NEXT pair's scores: at
            # iteration start DVE/ACT must turn around the new pair's
            # block-0 stats immediately (the 2-bank score PSUM recycle
            # depends on them) and finalize work there stalls the PE ~2us.
            tofin = list(sts)
            for t_hi in (9, 7, 5, 3, 1):
                sts = [new_stage(t_hi), new_stage(t_hi - 1)]
                for b in range(NBLK):
                    if b == 2:
                        for st in tofin:
                            emit_finalize(st)
                        tofin = []
                    if b == NBLK - 1:
                        pre_pv(pending[0])
                    for st in sts:
                        emit_score_block(st["t"], b, st)
                a = pending.pop(0)
                b2 = pending.pop(0)
                emit_pv(a, nxt=b2)
                emit_pv(b2, nxt=pending[0] if pending else None)
                tofin = sts
                pending.extend(sts)

            # tail: remaining PVs (last pair's finalizes slot in after the
            # first tail PV, long before their own PVs at the very end)
            for i, st in enumerate(pending):
                last = i + 1 == len(pending)
                emit_pv(st, nxt=None if last else pending[i + 1], final=last)
                if i == 0:
                    for st2 in tofin:
                        emit_finalize(st2)
                    tofin = []

    _split_multi_waits(nc)
    return nc


_NC_CACHE = None


def make_in_maps(Q, K, V):
    """Shard + permute + cast + pack the full inputs into per-core uploads."""
    n_seg = Q.shape[1] // SEG_LEN
    in_maps = []
    for c in range(8):
        b, g = divmod(c, n_seg)
        sl = slice(g * SEG_LEN, (g + 1) * SEG_LEN)
        QT = Q[b, sl].T[:, PERM].astype(np.float16)  # [1024, 2048]
        KT = K[b, sl].T[:, PERM].astype(np.float16)
        Vs = V[b, sl][PERM, :].astype(np.float16)  # [2048, 1024]
        in_maps.append(
            {
                # [128, 16, 8, 128]: partition r, q-tile t, d-chunk c, col j
                "QP": np.ascontiguousarray(
                    QT.reshape(NDCH, P, NKT, P).transpose(1, 2, 0, 3)
                ),
                # [128, 4, 8, 512]: partition r, k-block b, d-chunk c, col j
                "KP": np.ascontiguousarray(
                    KT.reshape(NDCH, P, NBLK, BL).transpose(1, 2, 0, 3)
                ),
                # [128, 16, 1024]: partition r, k-tile kt, d col
                "VP": np.ascontiguousarray(
                    Vs.reshape(NKT, P, D).transpose(1, 0, 2)
                ),
            }
        )
    return in_maps


def unshard(results, B, S, Dm):
    n_seg = S // SEG_LEN
    out = np.empty((B, S, Dm), dtype=np.float32)
    for c in range(8):
        b, g = divmod(c, n_seg)
        seg = np.empty((SEG_LEN, Dm), dtype=np.float32)
        seg[PERM, :] = results[c]["O"]
        out[b, g * SEG_LEN : (g + 1) * SEG_LEN, :] = seg
    return out


def kernel(Q, K, V):
    global _NC_CACHE
    Q = np.asarray(Q)
    K = np.asarray(K)
    V = np.asarray(V)
    B, S, Dm = Q.shape
    assert (B, S, Dm) == (2, 8192, 1024)

    if _NC_CACHE is None:
        _NC_CACHE = build_kernel()
    nc = _NC_CACHE

    res = run_bass_kernel_spmd(
        nc, make_in_maps(Q, K, V), core_ids=list(range(8))
    )
    return unshard(res.results, B, S, Dm)


if __name__ == "__main__":
    rng = np.random.default_rng(0)
    Q = rng.standard_normal((2, 8192, 1024), dtype=np.float32)
    K = rng.standard_normal((2, 8192, 1024), dtype=np.float32)
    V = rng.standard_normal((2, 8192, 1024), dtype=np.float32)
    out = kernel(Q=Q, K=K, V=V)
    print("ran ok", out.shape, out.dtype, np.abs(out).mean())
